# revision 18
# baseline (speedup 1.0000x reference)
# Bass/Trainium2 kernel for nn_L2PairwiceObjectiveFunction (pairwise L2 loss
# between per-row linear interpolations of two curve sets onto a common
# uniform grid).
#
# Full inputs: x, y1, y2 [1024, 8192] f32 (x sorted per row).
# Output: [1024, 1024] f32.
#
# Sharding: batch rows split across 8 NeuronCores (128 rows each, rows on
# SBUF partitions). The pairwise bilinear form uses a chunked AllGather of
# the transposed interpolated y2 grids (bf16) overlapped with second-half
# interpolation, followed by a local PE matmul.
#
# Interpolation: the common grid is UNIFORM, so each data point's grid cell
# is computable elementwise: c[j] = floor((x[j]-xmin)/dx) + 1. For grid
# point m the bracketing segment is the last j with c[j] <= m. Per segment
# the interpolant is linear in g: y(g_m) = A' + Bd*(m - c_j) where
# A' = y_j + B*(g_{c_j} - x_j) (value at the segment's own grid point),
# Bd = B*dx, B = dy/(gap+1e-9). We scatter int16-quantized (A', Bd) for y1
# and y2 (4 arrays) into grid bins with gpsimd local_scatter (per-partition
# indices; last-datum-per-bin dedup keeps indices unique), fill empty bins
# with a carry-forward tensor_tensor_scan, recover (m - c_j) as the carry
# "age" via a second scan form, then interpolate elementwise. Bin space is
# processed in two scatter halves x two scan/interp quarters to fit SBUF.
# Explicit deps keep DVE work out of LocalScatter windows (SBUF contention
# slows concurrent DVE ops ~10x).

import numpy as np

B, N, M, NCORES = 1024, 8192, 3000, 8
R = B // NCORES  # 128 rows per core
P = 128
NBINS = 3004        # bins (c in [0, 3001])
HBINS = 1502        # bins per scatter half: [0,1502), [1502,3004)
QW0 = 1024          # bins in first scan/interp quarter of each half
QW1 = HBINS - QW0   # bins in second quarter (478)
NIDXD = 4608        # default datum window per half (~11 sigma); the host
                    # passes exact per-half windows computed from the data
CPAD = N + 16       # padded cell-array width (need N+1 for shifted reads)
SA = 1489.0         # A' quantization scale (|A'| <= ~5.6 -> 8.3k of 16383)
AOFF = 16384.0      # A' offset so filled bins are nonzero (empty marker = 0)
SB = 2978.0         # Bd quantization scale (|Bd| clamped to 32725/SB ~ 10.99)
BCLIP = 32725.0
E6 = 1e6            # gap prescale so reciprocal input is ~[1e-3, 2e3]
KT = 24             # matmul k-tiles; grid padded 3000 -> 3072
KT1 = 11            # k-tiles coverable from bin half 0 (cols < 1408 <= 1501)
KT2 = 19            # k-tiles coverable after h1 quarter 0 (cols < 2432 <= 2525)
MT = KT * P
WB = 1024           # stage-A column block
NB = N // WB


def build_nc(xmin, xmax, nidx0=NIDXD, nidx1=NIDXD, debug=False):
    import concourse.bacc as bacc
    import concourse.mybir as mybir
    from concourse.tile import TileContext
    from concourse import library_config
    from concourse.tile_rust import add_dep_helper

    F32, BF16, I16 = mybir.dt.float32, mybir.dt.bfloat16, mybir.dt.int16
    FP8 = mybir.dt.float8e4
    A = mybir.AluOpType
    AF = mybir.ActivationFunctionType

    NIDXH = (nidx0, nidx1)
    NIDXMX = max(nidx0, nidx1)
    WOFF = (0, N - nidx1)
    dx = float((np.float32(xmax) - np.float32(xmin)) / np.float32(M - 1))
    inv_dx = float(np.float32(1.0) / np.float32(dx))

    nc = bacc.Bacc("TRN2", target_bir_lowering=False)
    x_in = nc.dram_tensor("x", [R, N], F32, kind="ExternalInput")
    y1_in = nc.dram_tensor("y1", [R, N], F32, kind="ExternalInput")
    y2_in = nc.dram_tensor("y2", [R, N], F32, kind="ExternalInput")
    xc_in = nc.dram_tensor("xc", [1, M], F32, kind="ExternalInput")
    id_in = nc.dram_tensor("ident", [P, P], BF16, kind="ExternalInput")
    o_out = nc.dram_tensor("out", [R, B], F32, kind="ExternalOutput")
    dbg = {}
    if debug:
        for nm, w, dt in [
            ("d_cp1", CPAD, I16), ("d_a1q", N, I16), ("d_b1q", N, I16),
            ("d_fa1", NBINS, F32), ("d_fb1", NBINS, F32),
            ("d_age", NBINS, F32),
            ("d_y1c", M, BF16), ("d_y2c", M, BF16), ("d_sq1", 1, F32),
        ]:
            dbg[nm] = nc.dram_tensor(nm, [R, w], dt, kind="ExternalOutput")

    with TileContext(nc) as tc:
        with (
            tc.tile_pool(name="pers", bufs=1) as pers,
            tc.tile_pool(name="psum", bufs=2, space="PSUM") as pp,
            tc.tile_pool(name="mmpsum", bufs=1, space="PSUM") as mmpp,
            tc.tile_pool(name="dram", bufs=1, space="DRAM") as dp,
        ):
            lib_bi = nc.gpsimd.load_library(library_config.local_scatter)

            x0 = pers.tile([P, 1], F32, tag="x0")
            xlast = pers.tile([P, 1], F32, tag="xlast")
            nc.sync.dma_start(out=x0[:], in_=x_in[:, 0:1])
            nc.sync.dma_start(out=xlast[:], in_=x_in[:, N - 1:N])

            y1c = pers.tile([P, MT], BF16, tag="y1c")
            y2c = pers.tile([P, MT], BF16, tag="y2c")
            nc.vector.memset(y1c[:, M:], 0)
            nc.vector.memset(y2c[:, M:], 0)
            ident = pers.tile([P, P], BF16, tag="ident")
            nc.sync.dma_start(out=ident[:], in_=id_in[:])
            y1T = pers.tile([P, MT], FP8, tag="y1T")
            y2T = pers.tile([P, MT], FP8, tag="y2T")
            sqacc = {}
            for ynm in ("y1", "y2"):
                s = pers.tile([P, 1], F32, tag=f"sqacc_{ynm}")
                nc.vector.memset(s[:], 0)
                sqacc[ynm] = s
            carries = {}   # latest scan carry [P,1] per array
            inits = {}     # scan initials from first datum
            ANAMES = ("age", "a1", "b1", "a2", "b2")
            cross = mmpp.tile([P, B], F32, space="PSUM")

            def transpose_tiles(k_lo, k_hi):
                last = None
                for kt in range(k_lo, k_hi):
                    for src, dstt in [(y1c, y1T), (y2c, y2T)]:
                        ps = pp.tile([P, P], BF16, tag="tps", space="PSUM")
                        nc.tensor.transpose(out=ps[:],
                                            in_=src[:, kt * P:(kt + 1) * P],
                                            identity=ident[:])
                        last = nc.vector.tensor_copy(
                            out=dstt[:, kt * P:(kt + 1) * P], in_=ps[:])
                return last

            def mm_chunk(rpool, agout, k_lo, k_hi):
                agv = agout[:].rearrange("(r p) f -> r p f", r=NCORES)
                for kt in range(k_lo, k_hi):
                    rhs = rpool.tile([P, B], FP8, tag="rhs")
                    lk = (kt - k_lo) * P
                    nc.sync.dma_start(
                        out=rhs[:].rearrange("p (r f) -> p r f", r=NCORES),
                        in_=agv[:, :, lk:lk + P].rearrange("r p f -> p r f"))
                    for jh in range(2):
                        nc.tensor.matmul(
                            cross[:, jh * 512:(jh + 1) * 512],
                            y1T[:, kt * P:(kt + 1) * P],
                            rhs[:, jh * 512:(jh + 1) * 512],
                            start=(kt == 0), stop=(kt == KT - 1))

            with (
                tc.tile_pool(name="qp", bufs=1) as qp,
                tc.tile_pool(name="ldp", bufs=2) as ldp,
                tc.tile_pool(name="sp", bufs=1) as sp,
                tc.tile_pool(name="rhsp", bufs=2) as rhsp,
            ):
                # ---- stage A: per-datum cells + interp coefficients ------
                cp1 = qp.tile([P, CPAD], I16, tag="cp1")   # c[j] + 1
                a1q = qp.tile([P, N], I16, tag="a1q")
                b1q = qp.tile([P, N], I16, tag="b1q")
                a2q = qp.tile([P, N], I16, tag="a2q")
                b2q = qp.tile([P, N], I16, tag="b2q")
                nc.vector.memset(cp1[:, N:], 30000)
                QARR = {"a1": a1q, "b1": b1q, "a2": a2q, "b2": b2q}

                for bi in range(NB):
                    lo = bi * WB
                    wext = WB + 1 if lo + WB < N else WB
                    sl = slice(lo, lo + WB)
                    xb = ldp.tile([P, WB + 1], F32, tag="xb")
                    yb1 = ldp.tile([P, WB + 1], F32, tag="yb1")
                    yb2 = ldp.tile([P, WB + 1], F32, tag="yb2")
                    nc.sync.dma_start(out=xb[:, :wext], in_=x_in[:, lo:lo + wext])
                    nc.sync.dma_start(out=yb1[:, :wext], in_=y1_in[:, lo:lo + wext])
                    nc.sync.dma_start(out=yb2[:, :wext], in_=y2_in[:, lo:lo + wext])
                    if wext == WB:
                        nc.vector.memset(xb[:, WB:], 0)
                        nc.vector.memset(yb1[:, WB:], 0)
                        nc.vector.memset(yb2[:, WB:], 0)
                    # t5 = (x - xmin)/dx + 1.5  (in [1.5, 3000.5] for real x)
                    t5 = sp.tile([P, WB], F32, tag="t5")
                    nc.scalar.activation(t5[:], xb[:, :WB], AF.Copy,
                                         bias=float(1.5 - xmin * inv_dx),
                                         scale=inv_dx)
                    # cp1 = round(t5) = c + 1
                    nc.vector.tensor_copy(out=cp1[:, sl], in_=t5[:])
                    cf = sp.tile([P, WB], F32, tag="cf")
                    nc.scalar.copy(out=cf[:], in_=cp1[:, sl])
                    gd = sp.tile([P, WB], F32, tag="gd")
                    nc.vector.tensor_tensor(out=gd[:], in0=cf[:], in1=t5[:],
                                            op=A.subtract)
                    # gx = (g_c - x) * 1e6 = (gd + 0.5) * dx * 1e6
                    gx = sp.tile([P, WB], BF16, tag="gxc")
                    nc.vector.tensor_scalar(out=gx[:], in0=gd[:],
                                            scalar1=0.5, scalar2=float(dx * E6),
                                            op0=A.add, op1=A.mult)
                    gap = sp.tile([P, WB], F32, tag="gap")
                    nc.vector.tensor_tensor(out=gap[:], in0=xb[:, 1:WB + 1],
                                            in1=xb[:, :WB], op=A.subtract)
                    # r6 = 1/(gap*1e6 + 1e-3) = recip(gap + 1e-9) * 1e-6
                    gape = sp.tile([P, WB], F32, tag="cf")  # cf dead
                    nc.scalar.activation(gape[:], gap[:], AF.Copy,
                                         bias=1e-3, scale=float(E6))
                    r6 = sp.tile([P, WB], F32, tag="r6")
                    nc.vector.reciprocal_approx_fast(out=r6[:], in_=gape[:])
                    r6c = sp.tile([P, WB], BF16, tag="r6c")
                    nc.scalar.copy(out=r6c[:], in_=r6[:])
                    for ynm, yb, aq, bq in (("1", yb1, a1q, b1q),
                                            ("2", yb2, a2q, b2q)):
                        ybc = sp.tile([P, WB + 1], BF16, tag="ybc")
                        nc.scalar.copy(out=ybc[:], in_=yb[:])
                        dy = sp.tile([P, WB], BF16, tag="dyc")
                        nc.vector.tensor_tensor(out=dy[:], in0=ybc[:, 1:WB + 1],
                                                in1=ybc[:, :WB], op=A.subtract)
                        bb = sp.tile([P, WB], BF16, tag="bbc")
                        nc.vector.tensor_tensor(out=bb[:], in0=dy[:], in1=r6c[:],
                                                op=A.mult)
                        # A' = y + B*gx
                        t6 = sp.tile([P, WB], BF16, tag="dyc")  # dy dead
                        nc.vector.tensor_tensor(out=t6[:], in0=bb[:], in1=gx[:],
                                                op=A.mult)
                        nc.vector.tensor_tensor(out=t6[:], in0=t6[:],
                                                in1=ybc[:, :WB], op=A.add)
                        nc.scalar.activation(aq[:, sl], t6[:], AF.Copy,
                                             bias=AOFF, scale=SA)
                        # Bd*SB = B*dx*SB, clamped to +-BCLIP
                        bt = sp.tile([P, WB], F32, tag="gap")  # dy dead
                        nc.vector.tensor_scalar(out=bt[:], in0=bb[:],
                                                scalar1=float(E6 * dx * SB),
                                                scalar2=BCLIP,
                                                op0=A.mult, op1=A.min)
                        nc.vector.tensor_scalar(
                            out=bq[:, sl], in0=bt[:], scalar1=-BCLIP,
                            scalar2=None, op0=A.max)

                # guard: one column per block -> reading guardA implies
                # every stage-A block's final DVE op has retired
                guardA = qp.tile([P, NB], I16, tag="guardA")
                for bi in range(NB):
                    nc.vector.tensor_copy(
                        out=guardA[:, bi:bi + 1],
                        in_=b2q[:, bi * WB + WB - 1:bi * WB + WB])
                inits["b1"] = 0.0
                inits["b2"] = 0.0
                inits["age"] = 0.0

                if debug:
                    nc.sync.dma_start(out=dbg["d_cp1"][:], in_=cp1[:])
                    nc.sync.dma_start(out=dbg["d_a1q"][:], in_=a1q[:])
                    nc.sync.dma_start(out=dbg["d_b1q"][:], in_=b1q[:])

                last_sc = None   # last scatter of previous half
                for h in range(2):
                    woff = WOFF[h]
                    nidx = NIDXH[h]
                    # ---- dedup + scatter index for this half -------------
                    neq_t = qp.tile([P, NIDXMX], I16, tag="neq")
                    neq = neq_t[:, :nidx]
                    if h == 1:
                        # touch: h1 dedup waits for h0's last interp (keeps
                        # DVE work out of the h0 scatter window); the column
                        # is overwritten by the real neq op below
                        nc.vector.tensor_copy(out=neq_t[:, 0:1],
                                              in_=sqacc["y2"][:, 0:1])
                    nc.vector.tensor_tensor(
                        out=neq, in0=cp1[:, woff:woff + nidx],
                        in1=cp1[:, woff + 1:woff + nidx + 1], op=A.not_equal)
                    if h == 1:
                        # second-to-last datum always survives; last never
                        nc.vector.memset(neq_t[:, nidx - 2:nidx - 1], 1)
                        nc.vector.memset(neq_t[:, nidx - 1:nidx], 0)
                    ix_t = qp.tile([P, NIDXMX], I16, tag="idx")
                    ix = ix_t[:, :nidx]
                    # touch: scatter release also waits on block 0's tail
                    nc.vector.tensor_copy(out=ix_t[:, 0:1],
                                          in_=guardA[:, 0:1])
                    nc.vector.tensor_tensor(out=ix, in0=neq,
                                            in1=cp1[:, woff:woff + nidx],
                                            op=A.mult)
                    if h == 0:
                        # bin = c = cp1 - 1; bins > 1501 pushed negative
                        nc.vector.tensor_scalar(out=ix, in0=ix, scalar1=1,
                                                scalar2=None, op0=A.subtract)
                        sel_t = qp.tile([P, NIDXMX], I16, tag="neq")
                        sel = sel_t[:, :nidx]
                        nc.vector.tensor_scalar(out=sel, in0=ix,
                                                scalar1=HBINS - 1, scalar2=None,
                                                op0=A.is_gt)
                        nc.vector.scalar_tensor_tensor(out=ix, in0=sel,
                                                       scalar=-32000.0,
                                                       in1=ix, op0=A.mult,
                                                       op1=A.add)
                    else:
                        # bin = c - 1502; bins < 1502 go negative (ignored)
                        nc.vector.tensor_scalar(out=ix, in0=ix,
                                                scalar1=HBINS + 1, scalar2=None,
                                                op0=A.subtract)

                    # ---- 4 scatters --------------------------------------
                    dsts = {}
                    for si, nm in enumerate(("a1", "b1", "a2", "b2")):
                        dst = qp.tile([P, HBINS], I16, tag=f"dst_{nm}")
                        sc_bi = nc.gpsimd.local_scatter(
                            dst[:], QARR[nm][:, woff:woff + nidx], ix,
                            channels=P, num_elems=HBINS, num_idxs=nidx)
                        add_dep_helper(sc_bi.ins, lib_bi.ins, sync=True,
                                       reason="lib before scatter")
                        dsts[nm] = dst
                        last_sc = sc_bi
                    if h == 0:
                        # scan initials from datum 0 (flat extension: B = 0)
                        # touch first: the copies wait for the scatter window
                        for nm, src in [("a1", a1q[:, 0:1]),
                                        ("a2", a2q[:, 0:1])]:
                            it = pers.tile([P, 1], F32, tag=f"init_{nm}")
                            nc.vector.tensor_copy(out=it[:],
                                                  in_=dsts["b2"][:, 0:1])
                            nc.vector.tensor_copy(out=it[:], in_=src)
                            inits[nm] = it

                    # ---- per quarter: fill scans + interpolation ---------
                    for qh, (qo, qw) in enumerate(((0, QW0), (QW0, QW1))):
                        qb0 = h * HBINS + qo
                        qs = slice(qo, qo + qw)
                        first = (h == 0 and qh == 0)
                        emt = sp.tile([P, WB], F32, tag="t5")
                        # touch: scans start only after the last scatter
                        nc.vector.tensor_copy(out=emt[:, 0:1],
                                              in_=dsts["b2"][:, 0:1])
                        nc.vector.tensor_scalar(
                            out=emt[:, :qw], in0=dsts["a1"][:, qs],
                            scalar1=0, scalar2=None, op0=A.is_equal)
                        filled = {}
                        FTAG = {"age": "cf", "a1": "gd", "b1": "gap",
                                "a2": "r6", "b2": "t6"}
                        for nm in ANAMES:
                            f = sp.tile([P, WB], F32, tag=FTAG[nm])
                            init = inits[nm] if first else carries[nm]
                            init_ap = (init if isinstance(init, float)
                                       else init[:, 0:1])
                            src = (emt[:, :qw] if nm == "age"
                                   else dsts[nm][:, qs])
                            nc.vector.tensor_tensor_scan(
                                f[:, :qw], emt[:, :qw], src, init_ap,
                                A.mult, A.add)
                            filled[nm] = f
                            cy = pers.tile([P, 1], F32, tag=f"carry_{nm}")
                            nc.vector.tensor_copy(
                                out=cy[:], in_=f[:, qw - 1:qw])
                            carries[nm] = cy

                        if debug:
                            for dnm, key in [("d_fa1", "a1"), ("d_fb1", "b1"),
                                             ("d_age", "age")]:
                                nc.sync.dma_start(
                                    out=dbg[dnm][:, qb0:qb0 + qw],
                                    in_=filled[key][:, :qw])

                        # interpolation over grid m in [qb0, min(qb0+qw, M))
                        W = min(qb0 + qw, M) - qb0
                        if W <= 0:
                            continue
                        fsl = slice(0, W)
                        xcb = sp.tile([P, WB], F32, tag="q7")
                        nc.sync.dma_start(
                            out=xcb[:, :W],
                            in_=xc_in[:, qb0:qb0 + W].to_broadcast([P, W]))
                        ma = sp.tile([P, WB], F32, tag="q8")
                        # touch: mask math stays out of the scatter window
                        nc.vector.tensor_copy(out=ma[:, 0:1],
                                              in_=dsts["b2"][:, 0:1])
                        nc.vector.tensor_scalar(
                            out=ma[:, :W], in0=xcb[:, :W],
                            scalar1=x0[:, 0:1], scalar2=None, op0=A.is_ge)
                        scr1 = sp.tile([P, WB], F32, tag="q9")
                        nc.vector.tensor_scalar(out=scr1[:, :W], in0=xcb[:, :W],
                                                scalar1=xlast[:, 0:1],
                                                scalar2=None, op0=A.is_le)
                        nc.vector.tensor_tensor(out=ma[:, :W], in0=ma[:, :W],
                                                in1=scr1[:, :W], op=A.mult)
                        for ynm, yc in (("1", y1c), ("2", y2c)):
                            # y = (fa - AOFF)/SA + (fb/SB)*age
                            t1 = sp.tile([P, WB], F32, tag="q9")
                            nc.vector.scalar_tensor_tensor(
                                out=t1[:, :W], in0=filled["b" + ynm][:, fsl],
                                scalar=float(1.0 / SB),
                                in1=filled["age"][:, fsl],
                                op0=A.mult, op1=A.mult)
                            t2 = sp.tile([P, WB], F32, tag="q7")
                            nc.vector.tensor_scalar(
                                out=t2[:, :W], in0=filled["a" + ynm][:, fsl],
                                scalar1=-AOFF, scalar2=float(1.0 / SA),
                                op0=A.add, op1=A.mult)
                            nc.vector.tensor_tensor(out=t2[:, :W],
                                                    in0=t2[:, :W],
                                                    in1=t1[:, :W], op=A.add)
                            nc.vector.tensor_tensor(out=yc[:, qb0:qb0 + W],
                                                    in0=t2[:, :W],
                                                    in1=ma[:, :W], op=A.mult)
                            spt = qp.tile([P, 1], F32, tag="spt")
                            e2 = sp.tile([P, WB], F32, tag="q9")
                            nc.scalar.activation(e2[:, :W], yc[:, qb0:qb0 + W],
                                                 AF.Square,
                                                 accum_out=spt[:, 0:1])
                            nc.vector.tensor_tensor(
                                out=sqacc["y" + ynm][:],
                                in0=sqacc["y" + ynm][:],
                                in1=spt[:], op=A.add)

                        if h == 1 and qh == 0:
                            # transposes for kt11..18 overlap h1 q1
                            transpose_tiles(KT1, KT2)

                    if h == 0:
                        # ---- overlap: transpose + AG + matmul of chunk 1 -
                        transpose_tiles(0, KT1)
                        agin1 = dp.tile([P, KT1 * P], FP8)
                        agout1 = dp.tile([NCORES * P, KT1 * P], FP8,
                                         addr_space="Shared")
                        nc.sync.dma_start(out=agin1[:], in_=y2T[:, 0:KT1 * P])
                        nc.gpsimd.collective_compute(
                            "AllGather", A.bypass,
                            replica_groups=[list(range(NCORES))],
                            ins=[agin1[:].opt()], outs=[agout1[:].opt()])
                        mm_chunk(rhsp, agout1, 0, KT1)

            # ---- sq = mean(y^2) ------------------------------------------
            sqa = {}
            for ynm in ("y1", "y2"):
                s = pers.tile([P, 1], F32, tag=f"sqa_{ynm}")
                nc.vector.tensor_scalar(out=s[:], in0=sqacc[ynm][:],
                                        scalar1=float(1.0 / M), scalar2=None,
                                        op0=A.mult)
                sqa[ynm] = s

            if debug:
                nc.sync.dma_start(out=dbg["d_y1c"][:], in_=y1c[:, 0:M])
                nc.sync.dma_start(out=dbg["d_y2c"][:], in_=y2c[:, 0:M])
                nc.sync.dma_start(out=dbg["d_sq1"][:], in_=sqa["y1"][:])

            with (
                tc.tile_pool(name="ep", bufs=1) as ep,
                tc.tile_pool(name="rhsp2", bufs=3) as rhsp2,
            ):
                # ---- transposes chunk 3 ----------------------------------
                transpose_tiles(KT2, KT)

                # ---- fold -1500*sq2[j] into spare matmul k-slots ---------
                # Slots m=3008..3011 (partition 64 of the last k-tile;
                # vector ops need partition base 0/32/64/96) carry a 4-term
                # fp8 residual cascade of v = -(M/2)/8*sq2; with lhs slots
                # = 8, cross picks up -1500*sq2[j], and (-2/M)*cross then
                # includes +sq2[j] -- no [B]-wide broadcast needed.
                NSL = 4
                sq2q = ep.tile([P, NSL], FP8, tag="sq2q")
                wres = ep.tile([P, 1], F32, tag="wres")
                qhf = ep.tile([P, 1], F32, tag="qhf")
                nc.vector.tensor_scalar(out=wres[:], in0=sqa["y2"][:],
                                        scalar1=float(-M / 16.0), scalar2=None,
                                        op0=A.mult)
                for k in range(NSL):
                    nc.vector.tensor_copy(out=sq2q[:, k:k + 1], in_=wres[:])
                    if k < NSL - 1:
                        nc.scalar.copy(out=qhf[:], in_=sq2q[:, k:k + 1])
                        nc.vector.tensor_tensor(out=wres[:], in0=wres[:],
                                                in1=qhf[:], op=A.subtract)
                scrd = dp.tile([P, NSL], FP8)
                nc.sync.dma_start(out=scrd[:], in_=sq2q[:])
                nc.sync.dma_start(out=y2T[64:64 + NSL, (KT - 1) * P:KT * P],
                                  in_=scrd[:].rearrange("i q -> q i"))
                nc.vector.memset(y1T[64:64 + NSL, (KT - 1) * P:KT * P], 8.0)

                # ---- AllGather chunk 2: kt11..23 (fp8) -------------------
                agin3 = dp.tile([P, (KT - KT1) * P], FP8)
                agout3 = dp.tile([NCORES * P, (KT - KT1) * P], FP8,
                                 addr_space="Shared")
                nc.sync.dma_start(out=agin3[:], in_=y2T[:, KT1 * P:MT])
                nc.gpsimd.collective_compute(
                    "AllGather", A.bypass,
                    replica_groups=[list(range(NCORES))],
                    ins=[agin3[:].opt()], outs=[agout3[:].opt()])

                # ---- matmul chunk 2 --------------------------------------
                mm_chunk(rhsp2, agout3, KT1, KT)

                # ---- epilogue --------------------------------------------
                diff = ep.tile([P, B], F32, tag="diff")
                nc.vector.tensor_scalar(out=diff[:], in0=cross[:],
                                        scalar1=float(-2.0 / M),
                                        scalar2=None, op0=A.mult)
                nc.vector.tensor_scalar(out=diff[:], in0=diff[:],
                                        scalar1=sqa["y1"][:, 0:1],
                                        scalar2=0.0, op0=A.add, op1=A.max)
                base = ep.tile([P, 1], F32, tag="base")
                nc.vector.tensor_tensor(out=base[:], in0=sqa["y1"][:],
                                        in1=sqa["y2"][:], op=A.add)
                nc.vector.tensor_scalar(out=base[:], in0=base[:], scalar1=1e-8,
                                        scalar2=None, op0=A.add)
                rbase = ep.tile([P, 1], F32, tag="rbase")
                nc.vector.reciprocal(rbase[:], base[:])
                nc.vector.scalar_tensor_tensor(out=diff[:], in0=diff[:],
                                               scalar=2.0,
                                               in1=rbase[:].to_broadcast([P, B]),
                                               op0=A.mult, op1=A.mult)
                lout = ep.tile([P, B], F32, tag="lout")
                nc.scalar.activation(lout[:], diff[:], AF.Sqrt)
                nc.sync.dma_start(out=o_out[:], in_=lout[:])

    nc.compile()
    return nc


def _host_prep(x):
    xmin = np.float32(x[:, 0].min())
    xmax = np.float32(x[:, -1].max())
    grid = np.linspace(np.float32(0.0), np.float32(1.0), M, dtype=np.float32)
    xc = (xmin + grid * (xmax - xmin)).astype(np.float32)[None, :]
    return xmin, xmax, xc


def kernel(x, y1, y2, debug=False, trace=False):
    import ml_dtypes
    from concourse.bass_utils import run_bass_kernel_spmd

    x = np.ascontiguousarray(x, dtype=np.float32)
    y1 = np.ascontiguousarray(y1, dtype=np.float32)
    y2 = np.ascontiguousarray(y2, dtype=np.float32)
    xmin, xmax, xc = _host_prep(x)
    ident = np.eye(P, dtype=ml_dtypes.bfloat16)

    # exact per-half scatter windows from the data (+margin, even)
    dxf = np.float32((xmax - xmin) / np.float32(M - 1))
    c = np.floor((x - xmin) / dxf).astype(np.int32) + 1
    n0 = int((c <= HBINS - 1).sum(axis=1).max())
    n1 = int((c >= HBINS).sum(axis=1).max())
    nidx0 = min(N, (n0 + 66) // 2 * 2)
    nidx1 = min(N, (n1 + 66) // 2 * 2)

    nc = build_nc(float(xmin), float(xmax), nidx0=nidx0, nidx1=nidx1,
                  debug=debug)
    in_maps = []
    for r in range(NCORES):
        rows = slice(r * R, (r + 1) * R)
        in_maps.append({"x": x[rows], "y1": y1[rows], "y2": y2[rows],
                        "xc": xc, "ident": ident})
    res = run_bass_kernel_spmd(nc, in_maps, core_ids=list(range(NCORES)),
                               trace=trace)
    out = np.concatenate([res.results[r]["out"] for r in range(NCORES)], axis=0)
    if debug or trace:
        return out, res
    return out


# revision 19
# speedup vs baseline: 1.0123x; 1.0123x over previous
# Bass/Trainium2 kernel for nn_L2PairwiceObjectiveFunction (pairwise L2 loss
# between per-row linear interpolations of two curve sets onto a common
# uniform grid).
#
# Full inputs: x, y1, y2 [1024, 8192] f32 (x sorted per row).
# Output: [1024, 1024] f32.
#
# Sharding: batch rows split across 8 NeuronCores (128 rows each, rows on
# SBUF partitions). The pairwise bilinear form uses a chunked AllGather of
# the transposed interpolated y2 grids (bf16) overlapped with second-half
# interpolation, followed by a local PE matmul.
#
# Interpolation: the common grid is UNIFORM, so each data point's grid cell
# is computable elementwise: c[j] = floor((x[j]-xmin)/dx) + 1. For grid
# point m the bracketing segment is the last j with c[j] <= m. Per segment
# the interpolant is linear in g: y(g_m) = A' + Bd*(m - c_j) where
# A' = y_j + B*(g_{c_j} - x_j) (value at the segment's own grid point),
# Bd = B*dx, B = dy/(gap+1e-9). We scatter int16-quantized (A', Bd) for y1
# and y2 (4 arrays) into grid bins with gpsimd local_scatter (per-partition
# indices; last-datum-per-bin dedup keeps indices unique), fill empty bins
# with a carry-forward tensor_tensor_scan, recover (m - c_j) as the carry
# "age" via a second scan form, then interpolate elementwise. Bin space is
# processed in two scatter halves x two scan/interp quarters to fit SBUF.
# Explicit deps keep DVE work out of LocalScatter windows (SBUF contention
# slows concurrent DVE ops ~10x).

import numpy as np

B, N, M, NCORES = 1024, 8192, 3000, 8
R = B // NCORES  # 128 rows per core
P = 128
NBINS = 3004        # bins (c in [0, 3001])
HBINS = 1502        # bins per scatter half: [0,1502), [1502,3004)
QW0 = 1024          # bins in first scan/interp quarter of each half
QW1 = HBINS - QW0   # bins in second quarter (478)
NIDXD = 4608        # default datum window per half (~11 sigma); the host
                    # passes exact per-half windows computed from the data
CPAD = N + 16       # padded cell-array width (need N+1 for shifted reads)
SA = 1489.0         # A' quantization scale (|A'| <= ~5.6 -> 8.3k of 16383)
AOFF = 16384.0      # A' offset so filled bins are nonzero (empty marker = 0)
SB = 2978.0         # Bd quantization scale (|Bd| clamped to 32725/SB ~ 10.99)
BCLIP = 32725.0
E6 = 1e6            # gap prescale so reciprocal input is ~[1e-3, 2e3]
KT = 24             # matmul k-tiles; grid padded 3000 -> 3072
KT1 = 11            # k-tiles coverable from bin half 0 (cols < 1408 <= 1501)
KT2 = 19            # k-tiles coverable after h1 quarter 0 (cols < 2432 <= 2525)
MT = KT * P
WB = 1024           # stage-A column block
NB = N // WB


def build_nc(xmin, xmax, nidx0=NIDXD, nidx1=NIDXD, debug=False):
    import concourse.bacc as bacc
    import concourse.mybir as mybir
    from concourse.tile import TileContext
    from concourse import library_config
    from concourse.tile_rust import add_dep_helper

    F32, BF16, I16 = mybir.dt.float32, mybir.dt.bfloat16, mybir.dt.int16
    FP8 = mybir.dt.float8e4
    A = mybir.AluOpType
    AF = mybir.ActivationFunctionType

    NIDXH = (nidx0, nidx1)
    NIDXMX = max(nidx0, nidx1)
    WOFF = (0, N - nidx1)
    dx = float((np.float32(xmax) - np.float32(xmin)) / np.float32(M - 1))
    inv_dx = float(np.float32(1.0) / np.float32(dx))

    nc = bacc.Bacc("TRN2", target_bir_lowering=False)
    x_in = nc.dram_tensor("x", [R, N], F32, kind="ExternalInput")
    y1_in = nc.dram_tensor("y1", [R, N], F32, kind="ExternalInput")
    y2_in = nc.dram_tensor("y2", [R, N], F32, kind="ExternalInput")
    xc_in = nc.dram_tensor("xc", [1, M], F32, kind="ExternalInput")
    id_in = nc.dram_tensor("ident", [P, P], BF16, kind="ExternalInput")
    o_out = nc.dram_tensor("out", [R, B], F32, kind="ExternalOutput")
    dbg = {}
    if debug:
        for nm, w, dt in [
            ("d_cp1", CPAD, I16), ("d_a1q", N, I16), ("d_b1q", N, I16),
            ("d_fa1", NBINS, F32), ("d_fb1", NBINS, F32),
            ("d_age", NBINS, F32),
            ("d_y1c", M, BF16), ("d_y2c", M, BF16), ("d_sq1", 1, F32),
        ]:
            dbg[nm] = nc.dram_tensor(nm, [R, w], dt, kind="ExternalOutput")

    with TileContext(nc) as tc:
        with (
            tc.tile_pool(name="pers", bufs=1) as pers,
            tc.tile_pool(name="psum", bufs=2, space="PSUM") as pp,
            tc.tile_pool(name="mmpsum", bufs=1, space="PSUM") as mmpp,
            tc.tile_pool(name="dram", bufs=1, space="DRAM") as dp,
        ):
            lib_bi = nc.gpsimd.load_library(library_config.local_scatter)

            x0 = pers.tile([P, 1], F32, tag="x0")
            xlast = pers.tile([P, 1], F32, tag="xlast")
            nc.sync.dma_start(out=x0[:], in_=x_in[:, 0:1])
            nc.sync.dma_start(out=xlast[:], in_=x_in[:, N - 1:N])

            y1c = pers.tile([P, MT], BF16, tag="y1c")
            y2c = pers.tile([P, MT], BF16, tag="y2c")
            nc.vector.memset(y1c[:, M:], 0)
            nc.vector.memset(y2c[:, M:], 0)
            ident = pers.tile([P, P], BF16, tag="ident")
            nc.sync.dma_start(out=ident[:], in_=id_in[:])
            y1T = pers.tile([P, MT], FP8, tag="y1T")
            y2T = pers.tile([P, MT], FP8, tag="y2T")
            sqacc = {}
            for ynm in ("y1", "y2"):
                s = pers.tile([P, 1], F32, tag=f"sqacc_{ynm}")
                nc.vector.memset(s[:], 0)
                sqacc[ynm] = s
            carries = {}   # latest scan carry [P,1] per array
            inits = {}     # scan initials from first datum
            ANAMES = ("age", "a1", "b1", "a2", "b2")
            cross = mmpp.tile([P, B], F32, space="PSUM")

            def transpose_tiles(k_lo, k_hi):
                last = None
                for kt in range(k_lo, k_hi):
                    for src, dstt in [(y1c, y1T), (y2c, y2T)]:
                        ps = pp.tile([P, P], BF16, tag="tps", space="PSUM")
                        nc.tensor.transpose(out=ps[:],
                                            in_=src[:, kt * P:(kt + 1) * P],
                                            identity=ident[:])
                        last = nc.vector.tensor_copy(
                            out=dstt[:, kt * P:(kt + 1) * P], in_=ps[:])
                return last

            def mm_chunk(rpool, agout, k_lo, k_hi):
                agv = agout[:].rearrange("(r p) f -> r p f", r=NCORES)
                for kt in range(k_lo, k_hi):
                    rhs = rpool.tile([P, B], FP8, tag="rhs")
                    lk = (kt - k_lo) * P
                    nc.sync.dma_start(
                        out=rhs[:].rearrange("p (r f) -> p r f", r=NCORES),
                        in_=agv[:, :, lk:lk + P].rearrange("r p f -> p r f"))
                    for jh in range(2):
                        nc.tensor.matmul(
                            cross[:, jh * 512:(jh + 1) * 512],
                            y1T[:, kt * P:(kt + 1) * P],
                            rhs[:, jh * 512:(jh + 1) * 512],
                            start=(kt == 0), stop=(kt == KT - 1))

            with (
                tc.tile_pool(name="qp", bufs=1) as qp,
                tc.tile_pool(name="ldp", bufs=2) as ldp,
                tc.tile_pool(name="sp", bufs=1) as sp,
                tc.tile_pool(name="rhsp", bufs=2) as rhsp,
            ):
                # ---- stage A: per-datum cells + interp coefficients ------
                cp1 = qp.tile([P, CPAD], I16, tag="cp1")   # c[j] + 1
                a1q = qp.tile([P, N], I16, tag="a1q")
                b1q = qp.tile([P, N], I16, tag="b1q")
                a2q = qp.tile([P, N], I16, tag="a2q")
                b2q = qp.tile([P, N], I16, tag="b2q")
                nc.vector.memset(cp1[:, N:], 30000)
                QARR = {"a1": a1q, "b1": b1q, "a2": a2q, "b2": b2q}

                for bi in range(NB):
                    lo = bi * WB
                    wext = WB + 1 if lo + WB < N else WB
                    sl = slice(lo, lo + WB)
                    xb = ldp.tile([P, WB + 1], F32, tag="xb")
                    yb1 = ldp.tile([P, WB + 1], F32, tag="yb1")
                    yb2 = ldp.tile([P, WB + 1], F32, tag="yb2")
                    nc.sync.dma_start(out=xb[:, :wext], in_=x_in[:, lo:lo + wext])
                    nc.sync.dma_start(out=yb1[:, :wext], in_=y1_in[:, lo:lo + wext])
                    nc.sync.dma_start(out=yb2[:, :wext], in_=y2_in[:, lo:lo + wext])
                    if wext == WB:
                        nc.vector.memset(xb[:, WB:], 0)
                        nc.vector.memset(yb1[:, WB:], 0)
                        nc.vector.memset(yb2[:, WB:], 0)
                    # t5 = (x - xmin)/dx + 1.5  (in [1.5, 3000.5] for real x)
                    t5 = sp.tile([P, WB], F32, tag="t5")
                    nc.scalar.activation(t5[:], xb[:, :WB], AF.Copy,
                                         bias=float(1.5 - xmin * inv_dx),
                                         scale=inv_dx)
                    # cp1 = round(t5) = c + 1
                    nc.vector.tensor_copy(out=cp1[:, sl], in_=t5[:])
                    cf = sp.tile([P, WB], F32, tag="cf")
                    nc.scalar.copy(out=cf[:], in_=cp1[:, sl])
                    gd = sp.tile([P, WB], F32, tag="gd")
                    nc.vector.tensor_tensor(out=gd[:], in0=cf[:], in1=t5[:],
                                            op=A.subtract)
                    # gx = (g_c - x) * 1e6 = (gd + 0.5) * dx * 1e6
                    gx = sp.tile([P, WB], BF16, tag="gxc")
                    nc.vector.tensor_scalar(out=gx[:], in0=gd[:],
                                            scalar1=0.5, scalar2=float(dx * E6),
                                            op0=A.add, op1=A.mult)
                    gap = sp.tile([P, WB], F32, tag="gap")
                    nc.vector.tensor_tensor(out=gap[:], in0=xb[:, 1:WB + 1],
                                            in1=xb[:, :WB], op=A.subtract)
                    # r6 = 1/(gap*1e6 + 1e-3) = recip(gap + 1e-9) * 1e-6
                    gape = sp.tile([P, WB], F32, tag="cf")  # cf dead
                    nc.scalar.activation(gape[:], gap[:], AF.Copy,
                                         bias=1e-3, scale=float(E6))
                    r6 = sp.tile([P, WB], F32, tag="r6")
                    nc.vector.reciprocal_approx_fast(out=r6[:], in_=gape[:])
                    r6c = sp.tile([P, WB], BF16, tag="r6c")
                    nc.scalar.copy(out=r6c[:], in_=r6[:])
                    for ynm, yb, aq, bq in (("1", yb1, a1q, b1q),
                                            ("2", yb2, a2q, b2q)):
                        ybc = sp.tile([P, WB + 1], BF16, tag="ybc")
                        nc.scalar.copy(out=ybc[:], in_=yb[:])
                        dy = sp.tile([P, WB], BF16, tag="dyc")
                        nc.vector.tensor_tensor(out=dy[:], in0=ybc[:, 1:WB + 1],
                                                in1=ybc[:, :WB], op=A.subtract)
                        bb = sp.tile([P, WB], BF16, tag="bbc")
                        nc.vector.tensor_tensor(out=bb[:], in0=dy[:], in1=r6c[:],
                                                op=A.mult)
                        # A' = y + B*gx
                        t6 = sp.tile([P, WB], BF16, tag="dyc")  # dy dead
                        nc.vector.tensor_tensor(out=t6[:], in0=bb[:], in1=gx[:],
                                                op=A.mult)
                        nc.vector.tensor_tensor(out=t6[:], in0=t6[:],
                                                in1=ybc[:, :WB], op=A.add)
                        nc.scalar.activation(aq[:, sl], t6[:], AF.Copy,
                                             bias=AOFF, scale=SA)
                        # Bd*SB = B*dx*SB, clamped to +-BCLIP
                        bt = sp.tile([P, WB], F32, tag="gap")  # dy dead
                        nc.vector.tensor_scalar(out=bt[:], in0=bb[:],
                                                scalar1=float(E6 * dx * SB),
                                                scalar2=BCLIP,
                                                op0=A.mult, op1=A.min)
                        nc.vector.tensor_scalar(
                            out=bq[:, sl], in0=bt[:], scalar1=-BCLIP,
                            scalar2=None, op0=A.max)

                # guard: one column per block -> reading guardA implies
                # every stage-A block's final DVE op has retired
                guardA = qp.tile([P, NB], I16, tag="guardA")
                for bi in range(NB):
                    nc.vector.tensor_copy(
                        out=guardA[:, bi:bi + 1],
                        in_=b2q[:, bi * WB + WB - 1:bi * WB + WB])
                inits["b1"] = 0.0
                inits["b2"] = 0.0
                inits["age"] = 0.0

                if debug:
                    nc.sync.dma_start(out=dbg["d_cp1"][:], in_=cp1[:])
                    nc.sync.dma_start(out=dbg["d_a1q"][:], in_=a1q[:])
                    nc.sync.dma_start(out=dbg["d_b1q"][:], in_=b1q[:])

                last_sc = None   # last scatter of previous half
                for h in range(2):
                    woff = WOFF[h]
                    nidx = NIDXH[h]
                    # ---- dedup + scatter index for this half -------------
                    neq_t = qp.tile([P, NIDXMX], I16, tag="neq")
                    neq = neq_t[:, :nidx]
                    if h == 1:
                        # touch: h1 dedup waits for h0's last interp (keeps
                        # DVE work out of the h0 scatter window); the column
                        # is overwritten by the real neq op below
                        nc.vector.tensor_copy(out=neq_t[:, 0:1],
                                              in_=sqacc["y2"][:, 0:1])
                    nc.vector.tensor_tensor(
                        out=neq, in0=cp1[:, woff:woff + nidx],
                        in1=cp1[:, woff + 1:woff + nidx + 1], op=A.not_equal)
                    if h == 1:
                        # second-to-last datum always survives; last never
                        nc.vector.memset(neq_t[:, nidx - 2:nidx - 1], 1)
                        nc.vector.memset(neq_t[:, nidx - 1:nidx], 0)
                    ix_t = qp.tile([P, NIDXMX], I16, tag="idx")
                    ix = ix_t[:, :nidx]
                    # touch: scatter release also waits on block 0's tail
                    nc.vector.tensor_copy(out=ix_t[:, 0:1],
                                          in_=guardA[:, 0:1])
                    nc.vector.tensor_tensor(out=ix, in0=neq,
                                            in1=cp1[:, woff:woff + nidx],
                                            op=A.mult)
                    if h == 0:
                        # bin = c = cp1 - 1; bins > 1501 pushed negative
                        nc.vector.tensor_scalar(out=ix, in0=ix, scalar1=1,
                                                scalar2=None, op0=A.subtract)
                        sel_t = qp.tile([P, NIDXMX], I16, tag="neq")
                        sel = sel_t[:, :nidx]
                        nc.vector.tensor_scalar(out=sel, in0=ix,
                                                scalar1=HBINS - 1, scalar2=None,
                                                op0=A.is_gt)
                        nc.vector.scalar_tensor_tensor(out=ix, in0=sel,
                                                       scalar=-32000.0,
                                                       in1=ix, op0=A.mult,
                                                       op1=A.add)
                    else:
                        # bin = c - 1502; bins < 1502 go negative (ignored)
                        nc.vector.tensor_scalar(out=ix, in0=ix,
                                                scalar1=HBINS + 1, scalar2=None,
                                                op0=A.subtract)

                    # ---- 4 scatters --------------------------------------
                    dsts = {}
                    for si, nm in enumerate(("a1", "b1", "a2", "b2")):
                        dst = qp.tile([P, HBINS], I16, tag=f"dst_{nm}")
                        sc_bi = nc.gpsimd.local_scatter(
                            dst[:], QARR[nm][:, woff:woff + nidx], ix,
                            channels=P, num_elems=HBINS, num_idxs=nidx)
                        add_dep_helper(sc_bi.ins, lib_bi.ins, sync=True,
                                       reason="lib before scatter")
                        dsts[nm] = dst
                        last_sc = sc_bi
                    if h == 0:
                        # scan initials from datum 0 (flat extension: B = 0)
                        # touch first: the copies wait for the scatter window
                        for nm, src in [("a1", a1q[:, 0:1]),
                                        ("a2", a2q[:, 0:1])]:
                            it = pers.tile([P, 1], F32, tag=f"init_{nm}")
                            nc.vector.tensor_copy(out=it[:],
                                                  in_=dsts["b2"][:, 0:1])
                            nc.vector.tensor_copy(out=it[:], in_=src)
                            inits[nm] = it

                    # ---- per quarter: fill scans + interpolation ---------
                    for qh, (qo, qw) in enumerate(((0, QW0), (QW0, QW1))):
                        qb0 = h * HBINS + qo
                        qs = slice(qo, qo + qw)
                        first = (h == 0 and qh == 0)
                        emt = sp.tile([P, WB], F32, tag="t5")
                        # touch: scans start only after the last scatter
                        nc.vector.tensor_copy(out=emt[:, 0:1],
                                              in_=dsts["b2"][:, 0:1])
                        nc.vector.tensor_scalar(
                            out=emt[:, :qw], in0=dsts["a1"][:, qs],
                            scalar1=0, scalar2=None, op0=A.is_equal)
                        filled = {}
                        FTAG = {"age": "cf", "a1": "gd", "b1": "gap",
                                "a2": "r6", "b2": "t6"}
                        for nm in ANAMES:
                            f = sp.tile([P, WB], F32, tag=FTAG[nm])
                            init = inits[nm] if first else carries[nm]
                            init_ap = (init if isinstance(init, float)
                                       else init[:, 0:1])
                            src = (emt[:, :qw] if nm == "age"
                                   else dsts[nm][:, qs])
                            nc.vector.tensor_tensor_scan(
                                f[:, :qw], emt[:, :qw], src, init_ap,
                                A.mult, A.add)
                            filled[nm] = f
                            cy = pers.tile([P, 1], F32, tag=f"carry_{nm}")
                            nc.vector.tensor_copy(
                                out=cy[:], in_=f[:, qw - 1:qw])
                            carries[nm] = cy

                        if debug:
                            for dnm, key in [("d_fa1", "a1"), ("d_fb1", "b1"),
                                             ("d_age", "age")]:
                                nc.sync.dma_start(
                                    out=dbg[dnm][:, qb0:qb0 + qw],
                                    in_=filled[key][:, :qw])

                        # interpolation over grid m in [qb0, min(qb0+qw, M))
                        W = min(qb0 + qw, M) - qb0
                        if W <= 0:
                            continue
                        fsl = slice(0, W)
                        xcb = sp.tile([P, WB], F32, tag="q7")
                        nc.sync.dma_start(
                            out=xcb[:, :W],
                            in_=xc_in[:, qb0:qb0 + W].to_broadcast([P, W]))
                        ma = sp.tile([P, WB], F32, tag="q8")
                        # touch: mask math stays out of the scatter window
                        nc.vector.tensor_copy(out=ma[:, 0:1],
                                              in_=dsts["b2"][:, 0:1])
                        nc.vector.tensor_scalar(
                            out=ma[:, :W], in0=xcb[:, :W],
                            scalar1=x0[:, 0:1], scalar2=None, op0=A.is_ge)
                        scr1 = sp.tile([P, WB], F32, tag="q9")
                        nc.vector.tensor_scalar(out=scr1[:, :W], in0=xcb[:, :W],
                                                scalar1=xlast[:, 0:1],
                                                scalar2=None, op0=A.is_le)
                        nc.vector.tensor_tensor(out=ma[:, :W], in0=ma[:, :W],
                                                in1=scr1[:, :W], op=A.mult)
                        for ynm, yc in (("1", y1c), ("2", y2c)):
                            # y = (fa - AOFF)/SA + (fb/SB)*age
                            t1 = sp.tile([P, WB], F32, tag="q9")
                            nc.vector.scalar_tensor_tensor(
                                out=t1[:, :W], in0=filled["b" + ynm][:, fsl],
                                scalar=float(1.0 / SB),
                                in1=filled["age"][:, fsl],
                                op0=A.mult, op1=A.mult)
                            t2 = sp.tile([P, WB], F32, tag="q7")
                            nc.vector.tensor_scalar(
                                out=t2[:, :W], in0=filled["a" + ynm][:, fsl],
                                scalar1=-AOFF, scalar2=float(1.0 / SA),
                                op0=A.add, op1=A.mult)
                            nc.vector.tensor_tensor(out=t2[:, :W],
                                                    in0=t2[:, :W],
                                                    in1=t1[:, :W], op=A.add)
                            nc.vector.tensor_tensor(out=yc[:, qb0:qb0 + W],
                                                    in0=t2[:, :W],
                                                    in1=ma[:, :W], op=A.mult)
                            spt = qp.tile([P, 1], F32, tag="spt")
                            e2 = sp.tile([P, WB], F32, tag="q9")
                            nc.scalar.activation(e2[:, :W], yc[:, qb0:qb0 + W],
                                                 AF.Square,
                                                 accum_out=spt[:, 0:1])
                            nc.vector.tensor_tensor(
                                out=sqacc["y" + ynm][:],
                                in0=sqacc["y" + ynm][:],
                                in1=spt[:], op=A.add)

                        if h == 1 and qh == 0:
                            # ---- overlap: chunk 2a while h1 q1 runs ------
                            transpose_tiles(KT1, KT2)
                            agin2 = dp.tile([P, (KT2 - KT1) * P], FP8)
                            agout2 = dp.tile([NCORES * P, (KT2 - KT1) * P],
                                             FP8, addr_space="Shared")
                            nc.sync.dma_start(out=agin2[:],
                                              in_=y2T[:, KT1 * P:KT2 * P])
                            nc.gpsimd.collective_compute(
                                "AllGather", A.bypass,
                                replica_groups=[list(range(NCORES))],
                                ins=[agin2[:].opt()], outs=[agout2[:].opt()])
                            mm_chunk(rhsp, agout2, KT1, KT2)

                    if h == 0:
                        # ---- overlap: transpose + AG + matmul of chunk 1 -
                        transpose_tiles(0, KT1)
                        agin1 = dp.tile([P, KT1 * P], FP8)
                        agout1 = dp.tile([NCORES * P, KT1 * P], FP8,
                                         addr_space="Shared")
                        nc.sync.dma_start(out=agin1[:], in_=y2T[:, 0:KT1 * P])
                        nc.gpsimd.collective_compute(
                            "AllGather", A.bypass,
                            replica_groups=[list(range(NCORES))],
                            ins=[agin1[:].opt()], outs=[agout1[:].opt()])
                        mm_chunk(rhsp, agout1, 0, KT1)

            # ---- sq = mean(y^2) ------------------------------------------
            sqa = {}
            for ynm in ("y1", "y2"):
                s = pers.tile([P, 1], F32, tag=f"sqa_{ynm}")
                nc.vector.tensor_scalar(out=s[:], in0=sqacc[ynm][:],
                                        scalar1=float(1.0 / M), scalar2=None,
                                        op0=A.mult)
                sqa[ynm] = s

            if debug:
                nc.sync.dma_start(out=dbg["d_y1c"][:], in_=y1c[:, 0:M])
                nc.sync.dma_start(out=dbg["d_y2c"][:], in_=y2c[:, 0:M])
                nc.sync.dma_start(out=dbg["d_sq1"][:], in_=sqa["y1"][:])

            with (
                tc.tile_pool(name="ep", bufs=1) as ep,
                tc.tile_pool(name="rhsp2", bufs=3) as rhsp2,
            ):
                # ---- transposes chunk 3 ----------------------------------
                transpose_tiles(KT2, KT)

                # ---- fold -1500*sq2[j] into spare matmul k-slots ---------
                # Slots m=3008..3011 (partition 64 of the last k-tile;
                # vector ops need partition base 0/32/64/96) carry a 4-term
                # fp8 residual cascade of v = -(M/2)/8*sq2; with lhs slots
                # = 8, cross picks up -1500*sq2[j], and (-2/M)*cross then
                # includes +sq2[j] -- no [B]-wide broadcast needed.
                NSL = 4
                sq2q = ep.tile([P, NSL], FP8, tag="sq2q")
                wres = ep.tile([P, 1], F32, tag="wres")
                qhf = ep.tile([P, 1], F32, tag="qhf")
                nc.vector.tensor_scalar(out=wres[:], in0=sqa["y2"][:],
                                        scalar1=float(-M / 16.0), scalar2=None,
                                        op0=A.mult)
                for k in range(NSL):
                    nc.vector.tensor_copy(out=sq2q[:, k:k + 1], in_=wres[:])
                    if k < NSL - 1:
                        nc.scalar.copy(out=qhf[:], in_=sq2q[:, k:k + 1])
                        nc.vector.tensor_tensor(out=wres[:], in0=wres[:],
                                                in1=qhf[:], op=A.subtract)
                scrd = dp.tile([P, NSL], FP8)
                nc.sync.dma_start(out=scrd[:], in_=sq2q[:])
                nc.sync.dma_start(out=y2T[64:64 + NSL, (KT - 1) * P:KT * P],
                                  in_=scrd[:].rearrange("i q -> q i"))
                nc.vector.memset(y1T[64:64 + NSL, (KT - 1) * P:KT * P], 8.0)

                # ---- AllGather chunk 3 (fp8) -----------------------------
                agin3 = dp.tile([P, (KT - KT2) * P], FP8)
                agout3 = dp.tile([NCORES * P, (KT - KT2) * P], FP8,
                                 addr_space="Shared")
                nc.sync.dma_start(out=agin3[:], in_=y2T[:, KT2 * P:MT])
                nc.gpsimd.collective_compute(
                    "AllGather", A.bypass,
                    replica_groups=[list(range(NCORES))],
                    ins=[agin3[:].opt()], outs=[agout3[:].opt()])

                # ---- matmul chunk 3 --------------------------------------
                mm_chunk(rhsp2, agout3, KT2, KT)

                # ---- epilogue --------------------------------------------
                diff = ep.tile([P, B], F32, tag="diff")
                nc.vector.tensor_scalar(out=diff[:], in0=cross[:],
                                        scalar1=float(-2.0 / M),
                                        scalar2=None, op0=A.mult)
                nc.vector.tensor_scalar(out=diff[:], in0=diff[:],
                                        scalar1=sqa["y1"][:, 0:1],
                                        scalar2=0.0, op0=A.add, op1=A.max)
                base = ep.tile([P, 1], F32, tag="base")
                nc.vector.tensor_tensor(out=base[:], in0=sqa["y1"][:],
                                        in1=sqa["y2"][:], op=A.add)
                nc.vector.tensor_scalar(out=base[:], in0=base[:], scalar1=1e-8,
                                        scalar2=None, op0=A.add)
                rbase = ep.tile([P, 1], F32, tag="rbase")
                nc.vector.reciprocal(rbase[:], base[:])
                nc.vector.scalar_tensor_tensor(out=diff[:], in0=diff[:],
                                               scalar=2.0,
                                               in1=rbase[:].to_broadcast([P, B]),
                                               op0=A.mult, op1=A.mult)
                lout = ep.tile([P, B], F32, tag="lout")
                nc.scalar.activation(lout[:], diff[:], AF.Sqrt)
                nc.sync.dma_start(out=o_out[:], in_=lout[:])

    nc.compile()
    return nc


def _host_prep(x):
    xmin = np.float32(x[:, 0].min())
    xmax = np.float32(x[:, -1].max())
    grid = np.linspace(np.float32(0.0), np.float32(1.0), M, dtype=np.float32)
    xc = (xmin + grid * (xmax - xmin)).astype(np.float32)[None, :]
    return xmin, xmax, xc


def kernel(x, y1, y2, debug=False, trace=False):
    import ml_dtypes
    from concourse.bass_utils import run_bass_kernel_spmd

    x = np.ascontiguousarray(x, dtype=np.float32)
    y1 = np.ascontiguousarray(y1, dtype=np.float32)
    y2 = np.ascontiguousarray(y2, dtype=np.float32)
    xmin, xmax, xc = _host_prep(x)
    ident = np.eye(P, dtype=ml_dtypes.bfloat16)

    # exact per-half scatter windows from the data (+margin, even)
    dxf = np.float32((xmax - xmin) / np.float32(M - 1))
    c = np.floor((x - xmin) / dxf).astype(np.int32) + 1
    n0 = int((c <= HBINS - 1).sum(axis=1).max())
    n1 = int((c >= HBINS).sum(axis=1).max())
    nidx0 = min(N, (n0 + 66) // 2 * 2)
    nidx1 = min(N, (n1 + 66) // 2 * 2)

    nc = build_nc(float(xmin), float(xmax), nidx0=nidx0, nidx1=nidx1,
                  debug=debug)
    in_maps = []
    for r in range(NCORES):
        rows = slice(r * R, (r + 1) * R)
        in_maps.append({"x": x[rows], "y1": y1[rows], "y2": y2[rows],
                        "xc": xc, "ident": ident})
    res = run_bass_kernel_spmd(nc, in_maps, core_ids=list(range(NCORES)),
                               trace=trace)
    out = np.concatenate([res.results[r]["out"] for r in range(NCORES)], axis=0)
    if debug or trace:
        return out, res
    return out
